# revision 1
# baseline (speedup 1.0000x reference)
"""Trainium2 Bass kernel for the two-level Haar-DWT detail (L1) loss.

Strategy (pure data parallel over batch, 8 NeuronCores):
  - Each core gets 4 of the 32 batch images (both `output` and `target`),
    viewed as a [6144, 512] row matrix; 24 pair-tiles of [128, 1024]
    (512 KiB loads).
  - The loss is linear until the per-band |.|; the (x+1)/2 normalization
    of both inputs only scales d = output - target by 0.5 (host-folded).
  - d = o - t: the host supplies -t; for half the pairs the DMA itself
    computes d (plain o load, then an SWDGE transfer of -t with
    accum_op=add, the CCE inline adder); the other half load o and -t
    separately and add on the VectorEngine.  Interleaving the two kinds
    keeps both the DMA chain and the VectorEngine busy.
  - The VectorEngine forms the level-1 column pair-combines with grouped
    [even-pairs | odd-pairs] bf16 outputs, so the level-2 column combines
    are contiguous-half adds at the bf16 2x rate.  Column permutations
    are free: every band ends in an abs-sum.
  - The TensorEngine (bf16) folds all row pair-combines, pair-merged
    into 5 wide matmuls per tile-pair; the 0.1 LL1 weight is baked into
    the weights (q = bf16(0.1)).
  - All band blocks of a pair land in one 3-bank PSUM region; two
    ScalarEngine Abs-activations with accum_out produce per-partition
    abs-sums with exactly the relative weights the loss needs.
  - Each core emits [128, 4]; host combines in float64.
"""

import numpy as np

B, C, H, W = 32, 3, 512, 512
N_CORES = 8
B_PER_CORE = B // N_CORES
ROWS = B_PER_CORE * C * H  # 6144
COLS = W  # 512
NB = 4  # row-blocks per DMA super-tile (1 MiB loads)
NT = ROWS // 128  # 48 tiles per core
NG = ROWS // (128 * NB)  # 12 super-tiles
NP = NT // 2  # 24 tile-pairs

_CACHE = {}


def _make_weights():
    import ml_dtypes
    q = ml_dtypes.bfloat16(0.1)  # LL1 loss weight, baked into W1q
    # w1q[k, m]: row pair-combine for the S (col-sum) path.
    # m<64: +q at rows 2m, 2m+1 (pair sum -> LL1, pre-weighted);
    # m=64+mm: -1/+1 (pair diff -> LH1).
    w1q = np.zeros((128, 128), ml_dtypes.bfloat16)
    w1 = np.zeros((128, 128), ml_dtypes.bfloat16)
    for m in range(64):
        w1q[2 * m, m] = q
        w1q[2 * m + 1, m] = q
        w1q[2 * m, 64 + m] = -1.0
        w1q[2 * m + 1, 64 + m] = 1.0
        # plain +-1 for the D (col-diff) path: HL1 | HH1
        w1[2 * m, m] = 1.0
        w1[2 * m + 1, m] = 1.0
        w1[2 * m, 64 + m] = -1.0
        w1[2 * m + 1, 64 + m] = 1.0
    # 4-row combines for level 2.
    w24s = np.zeros((128, 32), ml_dtypes.bfloat16)  # sums   -> HL2
    w24d = np.zeros((128, 32), ml_dtypes.bfloat16)  # diffs  -> HH2 | LH2
    for m in range(32):
        for r in range(4):
            w24s[4 * m + r, m] = 1.0
            w24d[4 * m + r, m] = -1.0 if r < 2 else 1.0
    return w1q, w1, w24s, w24d


def _build_bass():
    from contextlib import ExitStack

    import concourse.bacc as bacc
    import concourse.bass as bass
    import concourse.mybir as mybir
    import concourse.tile as tile

    F32 = mybir.dt.float32
    BF16 = mybir.dt.bfloat16
    X = mybir.AxisListType.X
    ADD = mybir.AluOpType.add
    ABS = mybir.ActivationFunctionType.Abs

    nc = bacc.Bacc("TRN2", target_bir_lowering=False, debug=False,
                   num_devices=N_CORES)
    o_d = nc.dram_tensor("o", [ROWS, COLS], F32, kind="ExternalInput").ap()
    t_d = nc.dram_tensor("tn", [ROWS, COLS], F32, kind="ExternalInput").ap()
    w1q_d = nc.dram_tensor("w1q", [128, 128], BF16, kind="ExternalInput").ap()
    w1_d = nc.dram_tensor("w1", [128, 128], BF16, kind="ExternalInput").ap()
    w24s_d = nc.dram_tensor("w24s", [128, 32], BF16, kind="ExternalInput").ap()
    w24d_d = nc.dram_tensor("w24d", [128, 32], BF16, kind="ExternalInput").ap()
    res_d = nc.dram_tensor("res", [128, 4], F32, kind="ExternalOutput").ap()

    # DRAM view for 512 KiB pair loads: [part, block, col] (the SBUF side
    # is one flat 4 KiB run per partition).
    def dram_view(ap, pr):
        return bass.AP(tensor=ap.tensor, offset=pr * 2 * 128 * COLS,
                       ap=[[COLS, 128], [128 * COLS, 2], [1, COLS]])

    with tile.TileContext(nc) as tc, ExitStack() as ctx:
        consts = ctx.enter_context(tc.tile_pool(name="consts", bufs=1))
        loads = ctx.enter_context(tc.tile_pool(name="loads", bufs=8))
        bands = ctx.enter_context(tc.tile_pool(name="bands", bufs=4))
        absout = ctx.enter_context(tc.tile_pool(name="absout", bufs=3))
        psP = ctx.enter_context(tc.tile_pool(name="psP", bufs=2, space="PSUM"))
        accp = ctx.enter_context(tc.tile_pool(name="accp", bufs=1))

        w1q_t = consts.tile([128, 128], BF16)
        w1_t = consts.tile([128, 128], BF16)
        w24s_t = consts.tile([128, 32], BF16)
        w24d_t = consts.tile([128, 32], BF16)
        nc.sync.dma_start(w1q_t[:], w1q_d)
        nc.sync.dma_start(w1_t[:], w1_d)
        nc.sync.dma_start(w24s_t[:], w24s_d)
        nc.sync.dma_start(w24d_t[:], w24d_d)

        acc1 = accp.tile([128, NP], F32)
        acc2 = accp.tile([128, NP], F32)
        mm = nc.tensor.matmul

        for pr in range(NP):
            if pr % 2 == 0:
                # Accum path: o then += (-t) via the DMA's inline adder
                # (SWDGE CCE).  The serial o->t chain of these pairs is
                # hidden by the independent-load pairs in between.
                ot = loads.tile([128, 2 * COLS], F32, tag="ot")
                ot3 = ot[:].rearrange("p (b c) -> p b c", b=2)
                nc.sync.dma_start(ot3, dram_view(o_d, pr))
                nc.gpsimd.dma_start(ot3, dram_view(t_d, pr), accum_op=ADD)
            else:
                # Independent loads; d = o + (-t) on the VectorEngine.
                o2 = loads.tile([128, 2 * COLS], F32, tag="o2")
                t2 = loads.tile([128, 2 * COLS], F32, tag="t2")
                nc.sync.dma_start(o2[:].rearrange("p (b c) -> p b c", b=2),
                                  dram_view(o_d, pr))
                nc.sync.dma_start(t2[:].rearrange("p (b c) -> p b c", b=2),
                                  dram_view(t_d, pr))
                ot = bands.tile([128, 2 * COLS], BF16, tag="d2")
                nc.vector.tensor_add(ot[:], o2[:], t2[:])

            psumP = psP.tile([128, 1536], F32)
            csP = bands.tile([128, 2, 256], BF16, tag="csP")
            cdP = bands.tile([128, 2, 256], BF16, tag="cdP")
            l2P = bands.tile([128, 2, 256], BF16, tag="l2P")
            for half in range(2):
                # level-1 column combines (stride-2 in, bf16 out)
                cs = csP[:, half, :]
                cd = cdP[:, half, :]
                dv = ot[:, half * COLS:(half + 1) * COLS]
                nc.vector.tensor_add(cs, dv[:, 0:COLS:2], dv[:, 1:COLS:2])
                nc.vector.tensor_sub(cd, dv[:, 1:COLS:2], dv[:, 0:COLS:2])
                # level-2 column combines (stride-2 bf16 in)
                nc.vector.tensor_add(l2P[:, half, 128:256],
                                     cs[:, 0:256:2], cs[:, 1:256:2])
                nc.vector.tensor_sub(l2P[:, half, 0:128],
                                     cs[:, 1:256:2], cs[:, 0:256:2])
            if True:

                # level-1 bands, pair-merged: psum b0 = S_A|S_B, b1 = D_A|D_B
                mm(psumP[:, 0:512], lhsT=w1q_t[:],
                   rhs=csP[:].rearrange("p a b -> p (a b)"),
                   start=True, stop=True)
                mm(psumP[:, 512:1024], lhsT=w1_t[:],
                   rhs=cdP[:].rearrange("p a b -> p (a b)"),
                   start=True, stop=True)
                # level-2 bands into psum bank 2, rows 0..95:
                #  [ 0:32]  diffs(A)  = HH2_A | LH2_A
                #  [32:64]  diffs(B)  = HH2_B | LH2_B
                #  [64:96]  sums(A|B) = HL2_A | HL2_B
                mm(psumP[0:32, 1024:1280], lhsT=w24d_t[:], rhs=l2P[:, 0, :],
                   start=True, stop=True)
                mm(psumP[32:64, 1024:1280], lhsT=w24d_t[:], rhs=l2P[:, 1, :],
                   start=True, stop=True)
                mm(psumP[64:96, 1024:1280], lhsT=w24s_t[:],
                   rhs=l2P[:, :, 0:128], start=True, stop=True)

                # Fused |.| + per-partition sums.
                ab1 = absout.tile([128, 1024], BF16, tag="ab1")
                ab2 = absout.tile([96, 256], BF16, tag="ab2")
                nc.scalar.activation(ab1[:], psumP[:, 0:1024], ABS,
                                     accum_out=acc1[:, pr:pr + 1])
                nc.scalar.activation(ab2[:], psumP[0:96, 1024:1280], ABS,
                                     accum_out=acc2[0:96, pr:pr + 1])

        res_t = accp.tile([128, 4], F32)
        nc.vector.memset(res_t[:], 0.0)
        nc.vector.tensor_reduce(res_t[:, 0:1], acc1[:], axis=X, op=ADD)
        nc.vector.tensor_reduce(res_t[0:96, 1:2], acc2[0:96, :], axis=X,
                                op=ADD)
        nc.sync.dma_start(res_d, res_t[:])

    nc.compile()
    return nc


def _get_bass():
    if "nc" not in _CACHE:
        _CACHE["nc"] = _build_bass()
    return _CACHE["nc"]


def _numpy_reference(output, target):
    """Full-precision fallback (only for the never-hit mixed-normalize case)."""
    o = output.astype(np.float64)
    t = target.astype(np.float64)
    if o.min() < 0:
        o = (o + 1.0) * 0.5
    if t.min() < 0:
        t = (t + 1.0) * 0.5

    def dwt(x):
        a = x[:, :, 0::2, 0::2]
        b = x[:, :, 0::2, 1::2]
        c = x[:, :, 1::2, 0::2]
        d = x[:, :, 1::2, 1::2]
        return (0.5 * (a + b + c + d), 0.5 * (-a - b + c + d),
                0.5 * (-a + b - c + d), 0.5 * (a - b - c + d))

    ll_o, lh_o, hl_o, hh_o = dwt(o)
    ll_t, lh_t, hl_t, hh_t = dwt(t)
    tot = (np.abs(lh_o - lh_t).mean() + np.abs(hl_o - hl_t).mean()
           + np.abs(hh_o - hh_t).mean() + 0.1 * np.abs(ll_o - ll_t).mean())
    _, lh2_o, hl2_o, hh2_o = dwt(ll_o)
    _, lh2_t, hl2_t, hh2_t = dwt(ll_t)
    tot += 0.5 * (np.abs(lh2_o - lh2_t).mean() + np.abs(hl2_o - hl2_t).mean()
                  + np.abs(hh2_o - hh2_t).mean())
    return np.float32(tot)


def _run_device(o, t, trace=False):
    """Shard [32,3,512,512] f32 arrays over 8 cores and run the Bass NEFF."""
    from concourse.bass_utils import run_bass_kernel_spmd

    nc = _get_bass()
    w1q, w1, w24s, w24d = _make_weights()
    tn = np.negative(t)  # device computes d = o + (-t) in the DMA
    in_maps = []
    for c in range(N_CORES):
        sl = slice(c * B_PER_CORE, (c + 1) * B_PER_CORE)
        in_maps.append({
            "o": o[sl].reshape(ROWS, COLS),
            "tn": tn[sl].reshape(ROWS, COLS),
            "w1q": w1q, "w1": w1, "w24s": w24s, "w24d": w24d,
        })
    res = run_bass_kernel_spmd(nc, in_maps, core_ids=list(range(N_CORES)),
                               trace=trace)
    _CACHE["last_result"] = res
    return res


def combine(results, both_norm=True):
    """Combine per-core [128, 4] abs-sum tensors into the scalar loss."""
    m = 0.0
    for r in results:
        v = r.astype(np.float64)
        m += v[:, 0].sum() + v[0:96, 1].sum()
    n1 = float(B * C * (H // 2) * (W // 2))
    scale = 4.0 * n1 if both_norm else 2.0 * n1
    return np.float32(m / scale)


def kernel(output, target):
    o = np.ascontiguousarray(np.asarray(output, dtype=np.float32))
    t = np.ascontiguousarray(np.asarray(target, dtype=np.float32))
    o_norm = bool(o.min() < 0.0)
    t_norm = bool(t.min() < 0.0)
    if o_norm != t_norm:
        # Normalization applied to only one input: the difference is no
        # longer a pure scale of o - t.  Practically unreachable for the
        # randn inputs this problem uses.
        return _numpy_reference(o, t)

    results = [r["res"] for r in _run_device(o, t).results]
    return combine(results, both_norm=o_norm)

